# revision 2
# baseline (speedup 1.0000x reference)
"""Chamfer loss (brute-force, no sigma) on 8 trn2 NeuronCores.

Strategy (data-parallel over batch, one batch element per core):
  negsq[m,n] = -|src_m - dst_n|^2 is produced by an augmented matmul
  (K=18 rows of exact bf16 splits) so PSUM holds the NEGATED squared
  distance; every reduction is then a MAX.  Per 128-row block the 4096
  PSUM columns are split across three evacuation lanes so no single
  engine is the pace-setter:
    - cols [0, DV):        DVE tensor_copy PSUM->SBUF bf16 (shipped)
    - cols [DV, 2048+A2):  ScalarE copies (two chunks, shipped)
    - cols [2048+A2, N):   DVE tensor_scalar PSUM->SBUF bf16 with fused
                           row-max accum (fwd done for free), then a 2x
                           tensor_tensor running max into bacc (bwd).
  The shipped prefix sb[:, :SHIP] is DMA-streamed to DRAM per block
  (the DMA engines are otherwise idle); the host finishes the fwd/bwd
  reductions for shipped columns, the 128-way cross-partition max for
  device columns, and the sqrt/means -- same division of labor as the
  old colneg path, just wider.
  Software-pipelined emission keeps the in-order DVE queue from
  head-of-line blocking: block i's chunk-0 copy is enqueued before
  block i-1's chunk-1-dependent ops.  Startup: GPSIMD memsets bacc
  while dummy matmuls warm the PE p-state ramp.
"""

import numpy as np
import ml_dtypes
from contextlib import ExitStack

B, C = 8, 3
M = N = 4096
NCORES = 8
PB = 128          # output partition block (m rows per matmul)
KAUG = 18         # augmented contraction dim
BIG = 3.0e4       # > max possible squared distance
MMN = 512         # matmul moving free dim (one fp32 PSUM bank)
PW = 2048         # psum chunk width (fp32, 4 banks)
DV = 896          # chunk-0 prefix evacuated by DVE tensor_copy (shipped)
A2 = 1280         # chunk-1 prefix evacuated by ScalarE (shipped)
SHIP = PW + A2    # shipped column count (DMA'd to DRAM, host-reduced)
DEV = N - SHIP    # device-reduced column count (fused fwd + TT bwd)
SB_BUFS = 3
VERSION = "shiplane-v17"

bf16np = ml_dtypes.bfloat16


# ----------------------------------------------------------------------------
# Device program
# ----------------------------------------------------------------------------

def _body(ctx, tc, lhs, rhs, rowneg_d, colneg_d, ship_d, m, n, reps=1):
    import concourse.mybir as mybir

    nc = tc.nc
    f32 = mybir.dt.float32
    bf16 = mybir.dt.bfloat16
    MAX = mybir.AluOpType.max

    nblk = m // PB

    cpool = ctx.enter_context(tc.tile_pool(name="const", bufs=1))
    ppool = ctx.enter_context(tc.tile_pool(name="psum", bufs=2, space="PSUM"))
    spool = ctx.enter_context(tc.tile_pool(name="sb", bufs=SB_BUFS))

    # Load block-0 stationary columns first so matmuls can start ASAP.
    lhs_t = cpool.tile([KAUG, m], bf16)
    nc.sync.dma_start(out=lhs_t[:, :PB], in_=lhs[:, :PB])
    rhs_t = cpool.tile([KAUG, n], bf16)
    nc.sync.dma_start(out=rhs_t[:, :PW], in_=rhs[:, :PW])
    nc.sync.dma_start(out=rhs_t[:, PW:], in_=rhs[:, PW:])
    nc.sync.dma_start(out=lhs_t[:, PB:], in_=lhs[:, PB:])

    facc = cpool.tile([PB, nblk], f32)
    bacc = cpool.tile([PB, DEV], bf16)
    nc.gpsimd.memset(bacc[:], -BIG)

    # Warm the PE p-state ramp while input DMAs land: tiny matmuls against a
    # memset tile so they have no DMA dependency.
    wconst = cpool.tile([KAUG, PB], bf16)
    nc.vector.memset(wconst[:], 0.0)
    wt = ppool.tile([PB, PW], f32, tag="pt")
    for _ in range(40):
        nc.tensor.matmul(wt[:, :1], wconst[:], wconst[:, :1],
                         start=True, stop=True)

    for rep in range(reps):
        for i in range(nblk):
            sb = spool.tile([PB, n], bf16, tag="sb")
            lhs_blk = lhs_t[:, i * PB:(i + 1) * PB]
            # ---- chunk 0 ----
            pt0 = ppool.tile([PB, PW], f32, tag="pt")
            for q in range(PW // MMN):
                nc.tensor.matmul(pt0[:, q * MMN:(q + 1) * MMN],
                                 lhs_blk, rhs_t[:, q * MMN:q * MMN + MMN],
                                 start=True, stop=True)
            # DVE evacuates the shipped prefix of chunk 0.
            nc.vector.tensor_copy(sb[:, :DV], pt0[:, :DV])
            # ScalarE evacuates the rest of chunk 0.
            nc.scalar.copy(sb[:, DV:PW], pt0[:, DV:])
            # ---- chunk 1 ----
            pt1 = ppool.tile([PB, PW], f32, tag="pt")
            for q in range(PW // MMN):
                n0 = PW + q * MMN
                nc.tensor.matmul(pt1[:, q * MMN:(q + 1) * MMN],
                                 lhs_blk, rhs_t[:, n0:n0 + MMN],
                                 start=True, stop=True)
            # ScalarE evacuates the shipped prefix of chunk 1.
            nc.scalar.copy(sb[:, PW:SHIP], pt1[:, :A2])
            # DVE evacuates the device range with fused fwd row-max accum.
            nc.vector.tensor_scalar(sb[:, SHIP:], pt1[:, A2:], float(-BIG),
                                    None, MAX, MAX,
                                    accum_out=facc[:, i:i + 1])
            # bwd running max over the device range (2x bf16 SBUF).
            nc.vector.tensor_tensor(bacc[:], bacc[:], sb[:, SHIP:], MAX)
            # Ship the host-reduced prefix to DRAM.
            nc.sync.dma_start(out=ship_d[:, i * SHIP:(i + 1) * SHIP],
                              in_=sb[:, :SHIP])

    nc.sync.dma_start(out=colneg_d[:], in_=bacc[:])
    nc.sync.dma_start(out=rowneg_d[:], in_=facc[:])


def build_nc(m=M, n=N, reps=1):
    import concourse.tile as tile
    import concourse.bacc as bacc_mod
    import concourse.mybir as mybir

    f32 = mybir.dt.float32
    bf16 = mybir.dt.bfloat16
    nblk = m // PB

    nc = bacc_mod.Bacc("TRN2", target_bir_lowering=False, debug=False)
    lhs = nc.dram_tensor("lhs_aug", [KAUG, m], bf16, kind="ExternalInput").ap()
    rhs = nc.dram_tensor("rhs_aug", [KAUG, n], bf16, kind="ExternalInput").ap()
    rowneg_d = nc.dram_tensor("rowneg", [PB, nblk], f32,
                              kind="ExternalOutput").ap()
    colneg_d = nc.dram_tensor("colneg", [PB, DEV], bf16,
                              kind="ExternalOutput").ap()
    ship_d = nc.dram_tensor("ship", [PB, nblk * SHIP], bf16,
                            kind="ExternalOutput").ap()
    with tile.TileContext(nc) as tc:
        with ExitStack() as ctx:
            _body(ctx, tc, lhs, rhs, rowneg_d, colneg_d, ship_d, m, n,
                  reps=reps)
    nc.compile()
    return nc


# ----------------------------------------------------------------------------
# Host-side input prep: exact bf16 splits for the augmented operands.
# The augmented product is the NEGATED squared distance:
#   -sq = 2*s.d - |s|^2 - |d|^2
# ----------------------------------------------------------------------------

def _split2(x):
    hi = x.astype(bf16np).astype(np.float64)
    lo = (x - hi).astype(bf16np).astype(np.float64)
    return hi, lo


def _split3(x):
    h = x.astype(bf16np).astype(np.float64)
    r = x - h
    mdl = r.astype(bf16np).astype(np.float64)
    l = (r - mdl).astype(bf16np).astype(np.float64)
    return h, mdl, l


def prep_inputs(pc_src, pc_dst):
    """Build per-batch augmented operands L, R: [B, 18, M/N] bf16."""
    s = np.asarray(pc_src, dtype=np.float64)   # [B, 3, M]
    d = np.asarray(pc_dst, dtype=np.float64)   # [B, 3, N]
    b = s.shape[0]
    m = s.shape[2]
    n = d.shape[2]

    s_hi, s_lo = _split2(s)
    d_hi, d_lo = _split2(d)
    s2 = ((s_hi + s_lo) ** 2).sum(axis=1)      # [B, M]
    d2 = ((d_hi + d_lo) ** 2).sum(axis=1)      # [B, N]
    s2h, s2m, s2l = _split3(-s2)
    d2h, d2m, d2l = _split3(-d2)

    L = np.zeros((b, KAUG, m), dtype=np.float64)
    R = np.zeros((b, KAUG, n), dtype=np.float64)
    L[:, 0:3] = 2.0 * s_hi
    R[:, 0:3] = d_hi
    L[:, 3:6] = 2.0 * s_hi
    R[:, 3:6] = d_lo
    L[:, 6:9] = 2.0 * s_lo
    R[:, 6:9] = d_hi
    L[:, 9:12] = 2.0 * s_lo
    R[:, 9:12] = d_lo
    L[:, 12:15] = 1.0
    R[:, 12] = d2h
    R[:, 13] = d2m
    R[:, 14] = d2l
    L[:, 15] = s2h
    L[:, 16] = s2m
    L[:, 17] = s2l
    R[:, 15:18] = 1.0
    return L.astype(bf16np), R.astype(bf16np)


# ----------------------------------------------------------------------------
# Cached PJRT runner (compile once, execute many)
# ----------------------------------------------------------------------------

_STATE = {}


def _get_runner(reps=1):
    key = (reps, VERSION, DV, A2, SB_BUFS)
    if key in _STATE:
        return _STATE[key]

    import jax
    from jax.experimental.shard_map import shard_map
    from jax.sharding import Mesh, PartitionSpec
    from concourse import bass2jax, mybir

    nc = build_nc(M, N, reps=reps)
    bass2jax.install_neuronx_cc_hook()

    in_names, out_names, out_avals = [], [], []
    for alloc in nc.m.functions[0].allocations:
        if not isinstance(alloc, mybir.MemoryLocationSet):
            continue
        name = alloc.memorylocations[0].name
        if alloc.kind == "ExternalInput":
            in_names.append(name)
        elif alloc.kind == "ExternalOutput":
            out_names.append(name)
            out_avals.append(jax.core.ShapedArray(
                tuple(alloc.tensor_shape), mybir.dt.np(alloc.dtype)))
    n_params = len(in_names)
    n_outs = len(out_names)
    all_in_names = tuple(in_names + out_names)
    donate = tuple(range(n_params, n_params + n_outs))

    def _jbody(*args):
        outs = bass2jax._bass_exec_p.bind(
            *args,
            out_avals=tuple(out_avals),
            in_names=all_in_names,
            out_names=tuple(out_names),
            lowering_input_output_aliases=(),
            sim_require_finite=True,
            sim_require_nnan=True,
            nc=nc,
        )
        return tuple(outs)

    devices = jax.devices()[:NCORES]
    mesh = Mesh(np.asarray(devices), ("core",))
    in_specs = (PartitionSpec("core"),) * (n_params + n_outs)
    out_specs = (PartitionSpec("core"),) * n_outs
    fn = jax.jit(
        shard_map(_jbody, mesh=mesh, in_specs=in_specs, out_specs=out_specs,
                  check_rep=False),
        donate_argnums=donate, keep_unused=True,
    )
    st = dict(fn=fn, nc=nc, in_names=in_names, out_names=out_names,
              out_avals=out_avals, n_params=n_params, zeros=None)
    _STATE[key] = st
    return st


def run_device(L, R, reps=1, _retry=True, fetch=True):
    """L, R: [NCORES, 18, M] bf16. Returns dict of full-shape outputs
    (rowneg [NCORES,128,32] fp32, colneg [NCORES,128,DEV] bf16,
    ship [NCORES,128,nblk*SHIP] bf16) holding negsq maxima / raw negsq;
    host still does the cross-partition maxes."""
    st = _get_runner(reps)
    concat_in = []
    for name in st["in_names"]:
        arr = L if name == "lhs_aug" else R
        concat_in.append(np.concatenate([arr[c] for c in range(NCORES)], axis=0))
    if st["zeros"] is None:
        st["zeros"] = [
            np.zeros((NCORES * av.shape[0], *av.shape[1:]), av.dtype)
            for av in st["out_avals"]
        ]
    try:
        out_arrs = st["fn"](*concat_in, *st["zeros"])
        if not fetch:
            for a in out_arrs:
                a.block_until_ready()
            return None
        out_np = [np.asarray(a) for a in out_arrs]
    except Exception:
        # The shared axon terminal occasionally reports a transient
        # device-unrecoverable state; it clears after a short pause.
        if not _retry:
            raise
        import time as _time
        _time.sleep(20.0)
        return run_device(L, R, reps=reps, _retry=False, fetch=fetch)
    outs = {}
    for i, name in enumerate(st["out_names"]):
        av = st["out_avals"][i]
        outs[name] = out_np[i].reshape(NCORES, *av.shape)
    return outs


# ----------------------------------------------------------------------------
# Public entry point
# ----------------------------------------------------------------------------

def _host_reduce(outs):
    # rowneg: [B, 128, nblk] fp32 -- fwd partial over device cols.
    # colneg: [B, 128, DEV] bf16 -- bwd partial (needs cross-partition max).
    # ship:   [B, 128, nblk*SHIP] bf16 -- raw negsq for shipped cols.
    # All hold -sq so min-sq = -max.
    nblk = M // PB
    rowneg = outs["rowneg"].astype(np.float32)            # [B,128,nblk]
    colneg = outs["colneg"].astype(np.float32)            # [B,128,DEV]
    ship = outs["ship"].reshape(B, PB, nblk, SHIP)        # bf16 view

    # fwd: per row (i, p): max over shipped cols + device partial.
    ship_f32 = np.asarray(ship, dtype=np.float32)
    fwd_ship = ship_f32.max(axis=3)                       # [B,128,nblk]
    rowmax = np.maximum(fwd_ship, rowneg)                 # [B,128,nblk]
    rowsq = np.maximum(-rowmax.astype(np.float64), 0.0)
    fwd = np.sqrt(rowsq).mean()

    # bwd: shipped cols reduce over all (p, i); device cols over p.
    bwd_ship = ship_f32.max(axis=(1, 2))                  # [B,SHIP]
    bwd_dev = colneg.max(axis=1)                          # [B,DEV]
    colmax = np.concatenate([bwd_ship, bwd_dev], axis=1)  # [B,N]
    colsq = np.maximum(-colmax.astype(np.float64), 0.0)
    bwd = np.sqrt(colsq).mean()

    total = np.float32(fwd + bwd)
    return total


def kernel(pc_src, pc_dst):
    L, R = prep_inputs(pc_src, pc_dst)
    outs = run_device(L, R)
    total = _host_reduce(outs)
    return (total, total, total)


# revision 17
# speedup vs baseline: 1.4755x; 1.4755x over previous
"""Chamfer loss (brute-force, no sigma) on 8 trn2 NeuronCores.

Strategy (data-parallel over batch, one batch element per core):
  negsq[m,n] = -|src_m - dst_n|^2 is produced by an augmented matmul
  (K=18 rows of exact bf16 splits) so PSUM holds the NEGATED squared
  distance; every reduction is then a MAX.  Per 128-row block the 4096
  PSUM columns are split across three evacuation lanes so no single
  engine is the pace-setter:
    - cols [0, DV):        DVE tensor_copy PSUM->SBUF bf16 (shipped)
    - cols [DV, 2048+A2):  ScalarE copies (two chunks, shipped)
    - cols [2048+A2, N):   DVE tensor_scalar PSUM->SBUF bf16 with fused
                           row-max accum (fwd done for free), then a 2x
                           tensor_tensor running max into bacc (bwd).
  The shipped prefix sb[:, :SHIP] is DMA-streamed to DRAM per block
  (the DMA engines are otherwise idle); the host finishes the fwd/bwd
  reductions for shipped columns, the 128-way cross-partition max for
  device columns, and the sqrt/means -- same division of labor as the
  old colneg path, just wider.
  Software-pipelined emission keeps the in-order DVE queue from
  head-of-line blocking: block i's chunk-0 copy is enqueued before
  block i-1's chunk-1-dependent ops.  Startup: GPSIMD memsets bacc
  while dummy matmuls warm the PE p-state ramp.
"""

import numpy as np
import ml_dtypes
from contextlib import ExitStack

B, C = 8, 3
M = N = 4096
NCORES = 8
PB = 128          # output partition block (m rows per matmul)
KAUG = 18         # augmented contraction dim
BIG = 3.0e4       # > max possible squared distance
MMN = 512         # matmul moving free dim (one fp32 PSUM bank)
PW = 2048         # psum chunk width (fp32, 4 banks)
# Per-block column partition: four PSUM tiles, each with exactly ONE
# compute reader (the tile framework chains multiple readers of a PSUM
# tile with cross-engine EventSemaphore waits, which serializes the
# reader lanes and was the v18/v19 bottleneck).  Bank-quantized sizes.
D_A = 1024        # ptA (2 banks): DVE tensor_scalar w/ fwd accum (+TT bwd)
D_B = 1536        # ptB (3 banks): Act copy (shipped)
D_C = 512         # ptC (1 bank):  DVE tensor_copy (shipped)
D_D = 1024        # ptD (2 banks): Act copy (shipped)
DEV = D_A         # device-reduced cols [0, DEV)
SHIP = N - DEV    # shipped column count (DMA'd to DRAM, host-reduced)
# sb staging layout: cross-engine writer ranges padded onto 1024-col
# (2KB) boundaries so no two writers on different engines share a
# subtile-tracking granule (B and D are both Act-written, so they sit
# adjacent and ship as ONE DMA).
S_TSP = 0                  # [0, 1024)      <- ptA  (DVE)
S_A0 = 1024                # [1024, 2560)   <- ptB  (Act)
S_A1 = 2560                # [2560, 3584)   <- ptD  (Act)
S_VC = 4096                # [4096, 4608)   <- ptC  (DVE)
SBW = S_VC + D_C           # staged sb tile width (4608 cols)
SB_BUFS = 4
VERSION = "shiplane-v26"

bf16np = ml_dtypes.bfloat16


# ----------------------------------------------------------------------------
# Device program
# ----------------------------------------------------------------------------

def _body(ctx, tc, lhs, rhs, rowneg_d, colneg_d, ship_d, m, n, reps=1):
    import concourse.mybir as mybir

    nc = tc.nc
    f32 = mybir.dt.float32
    bf16 = mybir.dt.bfloat16
    MAX = mybir.AluOpType.max

    nblk = m // PB

    cpool = ctx.enter_context(tc.tile_pool(name="const", bufs=1))
    pa = ctx.enter_context(tc.tile_pool(name="pa", bufs=1, space="PSUM"))
    pb = ctx.enter_context(tc.tile_pool(name="pb", bufs=1, space="PSUM"))
    pc = ctx.enter_context(tc.tile_pool(name="pc", bufs=1, space="PSUM"))
    pd = ctx.enter_context(tc.tile_pool(name="pd", bufs=1, space="PSUM"))
    spool = ctx.enter_context(tc.tile_pool(name="sb", bufs=SB_BUFS))

    # Load stationary columns split per matmul-lane so block 0's groups
    # unblock in PE order, spread across both HWDGE queues (SP/Act) so
    # descriptor generation pipelines instead of serializing on SP.
    lhs_t = cpool.tile([KAUG, m], bf16)
    rhs_t = cpool.tile([KAUG, n], bf16)
    nc.sync.dma_start(out=rhs_t[:, :D_A], in_=rhs[:, :D_A])          # A
    nc.scalar.dma_start(out=lhs_t[:, :PB], in_=lhs[:, :PB])
    nc.scalar.dma_start(out=rhs_t[:, D_A:D_A + D_B],
                        in_=rhs[:, D_A:D_A + D_B])                   # B
    nc.sync.dma_start(out=rhs_t[:, N - D_C:], in_=rhs[:, N - D_C:])  # C
    nc.sync.dma_start(out=rhs_t[:, D_A + D_B:N - D_C],
                      in_=rhs[:, D_A + D_B:N - D_C])                 # D
    nc.scalar.dma_start(out=lhs_t[:, PB:], in_=lhs[:, PB:])

    facc = cpool.tile([PB, nblk], f32)
    bacc = cpool.tile([PB, DEV], bf16)
    nc.gpsimd.memset(bacc[:], -BIG)

    # Warm the PE p-state ramp while input DMAs land: tiny matmuls against a
    # memset tile so they have no DMA dependency.
    wconst = cpool.tile([KAUG, PB], bf16)
    nc.vector.memset(wconst[:], 0.0)
    wt = pd.tile([PB, D_D], f32, tag="ptD")
    for _ in range(40):
        nc.tensor.matmul(wt[:, :1], wconst[:], wconst[:, :1],
                         start=True, stop=True)

    for rep in range(reps):
        for i in range(nblk):
            sb = spool.tile([PB, SBW], bf16, tag="sb")
            lhs_blk = lhs_t[:, i * PB:(i + 1) * PB]
            # ---- ptA: device range [0, D_A) -- DVE TSP w/ fwd accum ----
            ptA = pa.tile([PB, D_A], f32, tag="ptA")
            for q in range(D_A // MMN):
                nc.tensor.matmul(ptA[:, q * MMN:(q + 1) * MMN],
                                 lhs_blk, rhs_t[:, q * MMN:(q + 1) * MMN],
                                 start=True, stop=True)
            nc.vector.tensor_scalar(sb[:, S_TSP:S_TSP + D_A], ptA[:],
                                    float(-BIG), None, MAX, MAX,
                                    accum_out=facc[:, i:i + 1])
            # ---- ptB: cols [D_A, D_A+D_B) -- Act copy ----
            ptB = pb.tile([PB, D_B], f32, tag="ptB")
            for q in range(D_B // MMN):
                n0 = D_A + q * MMN
                nc.tensor.matmul(ptB[:, q * MMN:(q + 1) * MMN],
                                 lhs_blk, rhs_t[:, n0:n0 + MMN],
                                 start=True, stop=True)
            nc.scalar.copy(sb[:, S_A0:S_A0 + D_B], ptB[:])
            s0 = i * SHIP
            nc.sync.dma_start(out=ship_d[:, s0:s0 + D_B],
                              in_=sb[:, S_A0:S_A0 + D_B])
            # ---- ptC: cols [N-D_C, N) -- DVE copy ----
            ptC = pc.tile([PB, D_C], f32, tag="ptC")
            for q in range(D_C // MMN):
                n0 = N - D_C + q * MMN
                nc.tensor.matmul(ptC[:, q * MMN:(q + 1) * MMN],
                                 lhs_blk, rhs_t[:, n0:n0 + MMN],
                                 start=True, stop=True)
            nc.vector.tensor_copy(sb[:, S_VC:S_VC + D_C], ptC[:])
            nc.sync.dma_start(out=ship_d[:, s0 + D_B + D_D:s0 + SHIP],
                              in_=sb[:, S_VC:S_VC + D_C])
            # ---- ptD: cols [D_A+D_B, D_A+D_B+D_D) -- Act copy ----
            ptD = pd.tile([PB, D_D], f32, tag="ptD")
            for q in range(D_D // MMN):
                n0 = D_A + D_B + q * MMN
                nc.tensor.matmul(ptD[:, q * MMN:(q + 1) * MMN],
                                 lhs_blk, rhs_t[:, n0:n0 + MMN],
                                 start=True, stop=True)
            nc.scalar.copy(sb[:, S_A1:S_A1 + D_D], ptD[:])
            # bwd running max over the device range (2x bf16 SBUF); last on
            # the DVE queue so its RAW wait on the TSP hides behind Vcopy.
            nc.vector.tensor_tensor(bacc[:], bacc[:],
                                    sb[:, S_TSP:S_TSP + DEV], MAX)
            if rep == reps - 1 and i == nblk - 1:
                # Final outputs enter the DMA queue ahead of the last ship.
                nc.sync.dma_start(out=colneg_d[:], in_=bacc[:])
                nc.sync.dma_start(out=rowneg_d[:], in_=facc[:])
            # Ship the remaining host-reduced ranges to DRAM; each range
            # ships as soon as its own evacuation lands.
            nc.sync.dma_start(out=ship_d[:, s0 + D_B:s0 + D_B + D_D],
                              in_=sb[:, S_A1:S_A1 + D_D])




def build_nc(m=M, n=N, reps=1):
    import concourse.tile as tile
    import concourse.bacc as bacc_mod
    import concourse.mybir as mybir

    f32 = mybir.dt.float32
    bf16 = mybir.dt.bfloat16
    nblk = m // PB

    nc = bacc_mod.Bacc("TRN2", target_bir_lowering=False, debug=False)
    lhs = nc.dram_tensor("lhs_aug", [KAUG, m], bf16, kind="ExternalInput").ap()
    rhs = nc.dram_tensor("rhs_aug", [KAUG, n], bf16, kind="ExternalInput").ap()
    rowneg_d = nc.dram_tensor("rowneg", [PB, nblk], f32,
                              kind="ExternalOutput").ap()
    colneg_d = nc.dram_tensor("colneg", [PB, DEV], bf16,
                              kind="ExternalOutput").ap()
    ship_d = nc.dram_tensor("ship", [PB, nblk * SHIP], bf16,
                            kind="ExternalOutput").ap()
    with tile.TileContext(nc) as tc:
        with ExitStack() as ctx:
            _body(ctx, tc, lhs, rhs, rowneg_d, colneg_d, ship_d, m, n,
                  reps=reps)
    nc.compile()
    return nc


# ----------------------------------------------------------------------------
# Host-side input prep: exact bf16 splits for the augmented operands.
# The augmented product is the NEGATED squared distance:
#   -sq = 2*s.d - |s|^2 - |d|^2
# ----------------------------------------------------------------------------

def _split2(x):
    hi = x.astype(bf16np).astype(np.float64)
    lo = (x - hi).astype(bf16np).astype(np.float64)
    return hi, lo


def _split3(x):
    h = x.astype(bf16np).astype(np.float64)
    r = x - h
    mdl = r.astype(bf16np).astype(np.float64)
    l = (r - mdl).astype(bf16np).astype(np.float64)
    return h, mdl, l


def prep_inputs(pc_src, pc_dst):
    """Build per-batch augmented operands L, R: [B, 18, M/N] bf16."""
    s = np.asarray(pc_src, dtype=np.float64)   # [B, 3, M]
    d = np.asarray(pc_dst, dtype=np.float64)   # [B, 3, N]
    b = s.shape[0]
    m = s.shape[2]
    n = d.shape[2]

    s_hi, s_lo = _split2(s)
    d_hi, d_lo = _split2(d)
    s2 = ((s_hi + s_lo) ** 2).sum(axis=1)      # [B, M]
    d2 = ((d_hi + d_lo) ** 2).sum(axis=1)      # [B, N]
    s2h, s2m, s2l = _split3(-s2)
    d2h, d2m, d2l = _split3(-d2)

    L = np.zeros((b, KAUG, m), dtype=np.float64)
    R = np.zeros((b, KAUG, n), dtype=np.float64)
    L[:, 0:3] = 2.0 * s_hi
    R[:, 0:3] = d_hi
    L[:, 3:6] = 2.0 * s_hi
    R[:, 3:6] = d_lo
    L[:, 6:9] = 2.0 * s_lo
    R[:, 6:9] = d_hi
    L[:, 9:12] = 2.0 * s_lo
    R[:, 9:12] = d_lo
    L[:, 12:15] = 1.0
    R[:, 12] = d2h
    R[:, 13] = d2m
    R[:, 14] = d2l
    L[:, 15] = s2h
    L[:, 16] = s2m
    L[:, 17] = s2l
    R[:, 15:18] = 1.0
    return L.astype(bf16np), R.astype(bf16np)


# ----------------------------------------------------------------------------
# Cached PJRT runner (compile once, execute many)
# ----------------------------------------------------------------------------

_STATE = {}


def _get_runner(reps=1):
    key = (reps, VERSION, D_A, D_B, D_C, SB_BUFS)
    if key in _STATE:
        return _STATE[key]

    import jax
    from jax.experimental.shard_map import shard_map
    from jax.sharding import Mesh, PartitionSpec
    from concourse import bass2jax, mybir

    nc = build_nc(M, N, reps=reps)
    bass2jax.install_neuronx_cc_hook()

    in_names, out_names, out_avals = [], [], []
    for alloc in nc.m.functions[0].allocations:
        if not isinstance(alloc, mybir.MemoryLocationSet):
            continue
        name = alloc.memorylocations[0].name
        if alloc.kind == "ExternalInput":
            in_names.append(name)
        elif alloc.kind == "ExternalOutput":
            out_names.append(name)
            out_avals.append(jax.core.ShapedArray(
                tuple(alloc.tensor_shape), mybir.dt.np(alloc.dtype)))
    n_params = len(in_names)
    n_outs = len(out_names)
    all_in_names = tuple(in_names + out_names)
    donate = tuple(range(n_params, n_params + n_outs))

    def _jbody(*args):
        outs = bass2jax._bass_exec_p.bind(
            *args,
            out_avals=tuple(out_avals),
            in_names=all_in_names,
            out_names=tuple(out_names),
            lowering_input_output_aliases=(),
            sim_require_finite=True,
            sim_require_nnan=True,
            nc=nc,
        )
        return tuple(outs)

    devices = jax.devices()[:NCORES]
    mesh = Mesh(np.asarray(devices), ("core",))
    in_specs = (PartitionSpec("core"),) * (n_params + n_outs)
    out_specs = (PartitionSpec("core"),) * n_outs
    fn = jax.jit(
        shard_map(_jbody, mesh=mesh, in_specs=in_specs, out_specs=out_specs,
                  check_rep=False),
        donate_argnums=donate, keep_unused=True,
    )
    st = dict(fn=fn, nc=nc, in_names=in_names, out_names=out_names,
              out_avals=out_avals, n_params=n_params, zeros=None)
    _STATE[key] = st
    return st


def run_device(L, R, reps=1, _retry=True, fetch=True):
    """L, R: [NCORES, 18, M] bf16. Returns dict of full-shape outputs
    (rowneg [NCORES,128,32] fp32, colneg [NCORES,128,DEV] bf16,
    ship [NCORES,128,nblk*SHIP] bf16) holding negsq maxima / raw negsq;
    host still does the cross-partition maxes."""
    st = _get_runner(reps)
    concat_in = []
    for name in st["in_names"]:
        arr = L if name == "lhs_aug" else R
        concat_in.append(np.concatenate([arr[c] for c in range(NCORES)], axis=0))
    if st["zeros"] is None:
        st["zeros"] = [
            np.zeros((NCORES * av.shape[0], *av.shape[1:]), av.dtype)
            for av in st["out_avals"]
        ]
    try:
        out_arrs = st["fn"](*concat_in, *st["zeros"])
        if not fetch:
            for a in out_arrs:
                a.block_until_ready()
            return None
        out_np = [np.asarray(a) for a in out_arrs]
    except Exception:
        # The shared axon terminal occasionally reports a transient
        # device-unrecoverable state; it clears after a short pause.
        if not _retry:
            raise
        import time as _time
        _time.sleep(20.0)
        return run_device(L, R, reps=reps, _retry=False, fetch=fetch)
    outs = {}
    for i, name in enumerate(st["out_names"]):
        av = st["out_avals"][i]
        outs[name] = out_np[i].reshape(NCORES, *av.shape)
    return outs


# ----------------------------------------------------------------------------
# Public entry point
# ----------------------------------------------------------------------------

def _host_reduce(outs):
    # rowneg: [B, 128, nblk] fp32 -- fwd partial over device cols.
    # colneg: [B, 128, DEV] bf16 -- bwd partial (needs cross-partition max).
    # ship:   [B, 128, nblk*SHIP] bf16 -- raw negsq for shipped cols.
    # All hold -sq so min-sq = -max.
    nblk = M // PB
    rowneg = outs["rowneg"].astype(np.float32)            # [B,128,nblk]
    colneg = outs["colneg"].astype(np.float32)            # [B,128,DEV]
    ship = outs["ship"].reshape(B, PB, nblk, SHIP)        # bf16 view

    # fwd: per row (i, p): max over shipped cols + device partial.
    ship_f32 = np.asarray(ship, dtype=np.float32)
    fwd_ship = ship_f32.max(axis=3)                       # [B,128,nblk]
    rowmax = np.maximum(fwd_ship, rowneg)                 # [B,128,nblk]
    rowsq = np.maximum(-rowmax.astype(np.float64), 0.0)
    fwd = np.sqrt(rowsq).mean()

    # bwd: device cols [0, DEV) reduce over p; shipped cols over all (p, i).
    bwd_dev = colneg.max(axis=1)                          # [B,DEV]
    bwd_ship = ship_f32.max(axis=(1, 2))                  # [B,SHIP]
    colmax = np.concatenate([bwd_dev, bwd_ship], axis=1)  # [B,N]
    colsq = np.maximum(-colmax.astype(np.float64), 0.0)
    bwd = np.sqrt(colsq).mean()

    total = np.float32(fwd + bwd)
    return total


def kernel(pc_src, pc_dst):
    L, R = prep_inputs(pc_src, pc_dst)
    outs = run_device(L, R)
    total = _host_reduce(outs)
    return (total, total, total)


# revision 21
# speedup vs baseline: 1.5118x; 1.0246x over previous
"""Chamfer loss (brute-force, no sigma) on 8 trn2 NeuronCores.

Strategy (data-parallel over batch, one batch element per core):
  negsq[m,n] = -|src_m - dst_n|^2 is produced by an augmented matmul
  (K=18 rows of exact bf16 splits) so PSUM holds the NEGATED squared
  distance; every reduction is then a MAX.  Per 128-row block the 4096
  PSUM columns are split across three evacuation lanes so no single
  engine is the pace-setter:
    - cols [0, DV):        DVE tensor_copy PSUM->SBUF bf16 (shipped)
    - cols [DV, 2048+A2):  ScalarE copies (two chunks, shipped)
    - cols [2048+A2, N):   DVE tensor_scalar PSUM->SBUF bf16 with fused
                           row-max accum (fwd done for free), then a 2x
                           tensor_tensor running max into bacc (bwd).
  The shipped prefix sb[:, :SHIP] is DMA-streamed to DRAM per block
  (the DMA engines are otherwise idle); the host finishes the fwd/bwd
  reductions for shipped columns, the 128-way cross-partition max for
  device columns, and the sqrt/means -- same division of labor as the
  old colneg path, just wider.
  Software-pipelined emission keeps the in-order DVE queue from
  head-of-line blocking: block i's chunk-0 copy is enqueued before
  block i-1's chunk-1-dependent ops.  Startup: GPSIMD memsets bacc
  while dummy matmuls warm the PE p-state ramp.
"""

import numpy as np
import ml_dtypes
from contextlib import ExitStack

B, C = 8, 3
M = N = 4096
NCORES = 8
PB = 128          # output partition block (m rows per matmul)
KAUG = 18         # augmented contraction dim
BIG = 3.0e4       # > max possible squared distance
MMN = 512         # matmul moving free dim (one fp32 PSUM bank)
PW = 2048         # psum chunk width (fp32, 4 banks)
# Per-block column partition: four PSUM tiles, each with exactly ONE
# compute reader (the tile framework chains multiple readers of a PSUM
# tile with cross-engine EventSemaphore waits, which serializes the
# reader lanes and was the v18/v19 bottleneck).  Bank-quantized sizes.
D_A = 1024        # ptA (2 banks): DVE tensor_scalar w/ fwd accum (+TT bwd)
D_B = 1536        # ptB (3 banks): Act copy (shipped)
D_C = 512         # ptC (1 bank):  DVE tensor_copy (shipped)
D_D = 1024        # ptD (2 banks): Act copy (shipped)
DEV = D_A         # device-reduced cols [0, DEV)
SHIP = N - DEV    # shipped column count (DMA'd to DRAM, host-reduced)
# sb staging layout: cross-engine writer ranges padded onto 1024-col
# (2KB) boundaries so no two writers on different engines share a
# subtile-tracking granule (B and D are both Act-written, so they sit
# adjacent and ship as ONE DMA).
S_TSP = 0                  # [0, 1024)      <- ptA  (DVE)
S_A0 = 1024                # [1024, 2560)   <- ptB  (Act)
S_A1 = 2560                # [2560, 3584)   <- ptD  (Act)
S_VC = 4096                # [4096, 4608)   <- ptC  (DVE)
SBW = S_VC + D_C           # staged sb tile width (4608 cols)
SB_BUFS = 4
VERSION = "shiplane-v27"

bf16np = ml_dtypes.bfloat16


# ----------------------------------------------------------------------------
# Device program
# ----------------------------------------------------------------------------

def _body(ctx, tc, lhs, rhs, rowneg_d, colneg_d, ship_d, m, n, reps=1):
    import concourse.mybir as mybir

    nc = tc.nc
    f32 = mybir.dt.float32
    bf16 = mybir.dt.bfloat16
    MAX = mybir.AluOpType.max

    nblk = m // PB

    cpool = ctx.enter_context(tc.tile_pool(name="const", bufs=1))
    pa = ctx.enter_context(tc.tile_pool(name="pa", bufs=1, space="PSUM"))
    pb = ctx.enter_context(tc.tile_pool(name="pb", bufs=1, space="PSUM"))
    pc = ctx.enter_context(tc.tile_pool(name="pc", bufs=1, space="PSUM"))
    pd = ctx.enter_context(tc.tile_pool(name="pd", bufs=1, space="PSUM"))
    spool = ctx.enter_context(tc.tile_pool(name="sb", bufs=SB_BUFS))

    # Load stationary columns split per matmul-lane so block 0's groups
    # unblock in PE order, spread across both HWDGE queues (SP/Act) so
    # descriptor generation pipelines instead of serializing on SP.
    lhs_t = cpool.tile([KAUG, m], bf16)
    rhs_t = cpool.tile([KAUG, n], bf16)
    nc.sync.dma_start(out=rhs_t[:, :D_A], in_=rhs[:, :D_A])          # A
    nc.scalar.dma_start(out=lhs_t[:, :PB], in_=lhs[:, :PB])
    nc.scalar.dma_start(out=rhs_t[:, D_A:D_A + D_B],
                        in_=rhs[:, D_A:D_A + D_B])                   # B
    nc.sync.dma_start(out=rhs_t[:, N - D_C:], in_=rhs[:, N - D_C:])  # C
    nc.sync.dma_start(out=rhs_t[:, D_A + D_B:N - D_C],
                      in_=rhs[:, D_A + D_B:N - D_C])                 # D
    nc.scalar.dma_start(out=lhs_t[:, PB:], in_=lhs[:, PB:])

    facc = cpool.tile([PB, nblk], f32)
    bacc = cpool.tile([PB, DEV], bf16)
    nc.gpsimd.memset(bacc[:], -BIG)

    # Warm the PE p-state ramp while input DMAs land: tiny matmuls against a
    # memset tile so they have no DMA dependency.
    wconst = cpool.tile([KAUG, PB], bf16)
    nc.vector.memset(wconst[:], 0.0)
    wt = pd.tile([PB, D_D], f32, tag="ptD")
    for _ in range(40):
        nc.tensor.matmul(wt[:, :1], wconst[:], wconst[:, :1],
                         start=True, stop=True)

    for rep in range(reps):
        for i in range(nblk):
            sb = spool.tile([PB, SBW], bf16, tag="sb")
            lhs_blk = lhs_t[:, i * PB:(i + 1) * PB]
            # ---- ptA: device range [0, D_A) -- DVE TSP w/ fwd accum ----
            ptA = pa.tile([PB, D_A], f32, tag="ptA")
            for q in range(D_A // MMN):
                nc.tensor.matmul(ptA[:, q * MMN:(q + 1) * MMN],
                                 lhs_blk, rhs_t[:, q * MMN:(q + 1) * MMN],
                                 start=True, stop=True)
            nc.vector.tensor_scalar(sb[:, S_TSP:S_TSP + D_A], ptA[:],
                                    float(-BIG), None, MAX, MAX,
                                    accum_out=facc[:, i:i + 1])
            # ---- ptB: cols [D_A, D_A+D_B) -- Act copy ----
            ptB = pb.tile([PB, D_B], f32, tag="ptB")
            for q in range(D_B // MMN):
                n0 = D_A + q * MMN
                nc.tensor.matmul(ptB[:, q * MMN:(q + 1) * MMN],
                                 lhs_blk, rhs_t[:, n0:n0 + MMN],
                                 start=True, stop=True)
            nc.scalar.copy(sb[:, S_A0:S_A0 + D_B], ptB[:])
            s0 = i * SHIP
            last = (rep == reps - 1 and i == nblk - 1)
            # The final block's Act-dependent ships go out on the Act HWDGE
            # queue: descriptor generation starts the moment the copy
            # retires instead of crossing to the (possibly backlogged) SP
            # queue, shortening the end-of-kernel drain.
            shipq = nc.scalar if last else nc.sync
            shipq.dma_start(out=ship_d[:, s0:s0 + D_B],
                            in_=sb[:, S_A0:S_A0 + D_B])
            # ---- ptC: cols [N-D_C, N) -- DVE copy ----
            ptC = pc.tile([PB, D_C], f32, tag="ptC")
            for q in range(D_C // MMN):
                n0 = N - D_C + q * MMN
                nc.tensor.matmul(ptC[:, q * MMN:(q + 1) * MMN],
                                 lhs_blk, rhs_t[:, n0:n0 + MMN],
                                 start=True, stop=True)
            nc.vector.tensor_copy(sb[:, S_VC:S_VC + D_C], ptC[:])
            nc.sync.dma_start(out=ship_d[:, s0 + D_B + D_D:s0 + SHIP],
                              in_=sb[:, S_VC:S_VC + D_C])
            # ---- ptD: cols [D_A+D_B, D_A+D_B+D_D) -- Act copy ----
            ptD = pd.tile([PB, D_D], f32, tag="ptD")
            for q in range(D_D // MMN):
                n0 = D_A + D_B + q * MMN
                nc.tensor.matmul(ptD[:, q * MMN:(q + 1) * MMN],
                                 lhs_blk, rhs_t[:, n0:n0 + MMN],
                                 start=True, stop=True)
            nc.scalar.copy(sb[:, S_A1:S_A1 + D_D], ptD[:])
            shipq.dma_start(out=ship_d[:, s0 + D_B:s0 + D_B + D_D],
                            in_=sb[:, S_A1:S_A1 + D_D])
            # bwd running max over the device range (2x bf16 SBUF); last on
            # the DVE queue so its RAW wait on the TSP hides behind Vcopy.
            nc.vector.tensor_tensor(bacc[:], bacc[:],
                                    sb[:, S_TSP:S_TSP + DEV], MAX)
            if last:
                # Final outputs enter the DMA queue ahead of the last ship.
                nc.sync.dma_start(out=colneg_d[:], in_=bacc[:])
                nc.sync.dma_start(out=rowneg_d[:], in_=facc[:])





def build_nc(m=M, n=N, reps=1):
    import concourse.tile as tile
    import concourse.bacc as bacc_mod
    import concourse.mybir as mybir

    f32 = mybir.dt.float32
    bf16 = mybir.dt.bfloat16
    nblk = m // PB

    nc = bacc_mod.Bacc("TRN2", target_bir_lowering=False, debug=False)
    lhs = nc.dram_tensor("lhs_aug", [KAUG, m], bf16, kind="ExternalInput").ap()
    rhs = nc.dram_tensor("rhs_aug", [KAUG, n], bf16, kind="ExternalInput").ap()
    rowneg_d = nc.dram_tensor("rowneg", [PB, nblk], f32,
                              kind="ExternalOutput").ap()
    colneg_d = nc.dram_tensor("colneg", [PB, DEV], bf16,
                              kind="ExternalOutput").ap()
    ship_d = nc.dram_tensor("ship", [PB, nblk * SHIP], bf16,
                            kind="ExternalOutput").ap()
    with tile.TileContext(nc) as tc:
        with ExitStack() as ctx:
            _body(ctx, tc, lhs, rhs, rowneg_d, colneg_d, ship_d, m, n,
                  reps=reps)
    nc.compile()
    return nc


# ----------------------------------------------------------------------------
# Host-side input prep: exact bf16 splits for the augmented operands.
# The augmented product is the NEGATED squared distance:
#   -sq = 2*s.d - |s|^2 - |d|^2
# ----------------------------------------------------------------------------

def _split2(x):
    hi = x.astype(bf16np).astype(np.float64)
    lo = (x - hi).astype(bf16np).astype(np.float64)
    return hi, lo


def _split3(x):
    h = x.astype(bf16np).astype(np.float64)
    r = x - h
    mdl = r.astype(bf16np).astype(np.float64)
    l = (r - mdl).astype(bf16np).astype(np.float64)
    return h, mdl, l


def prep_inputs(pc_src, pc_dst):
    """Build per-batch augmented operands L, R: [B, 18, M/N] bf16."""
    s = np.asarray(pc_src, dtype=np.float64)   # [B, 3, M]
    d = np.asarray(pc_dst, dtype=np.float64)   # [B, 3, N]
    b = s.shape[0]
    m = s.shape[2]
    n = d.shape[2]

    s_hi, s_lo = _split2(s)
    d_hi, d_lo = _split2(d)
    s2 = ((s_hi + s_lo) ** 2).sum(axis=1)      # [B, M]
    d2 = ((d_hi + d_lo) ** 2).sum(axis=1)      # [B, N]
    s2h, s2m, s2l = _split3(-s2)
    d2h, d2m, d2l = _split3(-d2)

    L = np.zeros((b, KAUG, m), dtype=np.float64)
    R = np.zeros((b, KAUG, n), dtype=np.float64)
    L[:, 0:3] = 2.0 * s_hi
    R[:, 0:3] = d_hi
    L[:, 3:6] = 2.0 * s_hi
    R[:, 3:6] = d_lo
    L[:, 6:9] = 2.0 * s_lo
    R[:, 6:9] = d_hi
    L[:, 9:12] = 2.0 * s_lo
    R[:, 9:12] = d_lo
    L[:, 12:15] = 1.0
    R[:, 12] = d2h
    R[:, 13] = d2m
    R[:, 14] = d2l
    L[:, 15] = s2h
    L[:, 16] = s2m
    L[:, 17] = s2l
    R[:, 15:18] = 1.0
    return L.astype(bf16np), R.astype(bf16np)


# ----------------------------------------------------------------------------
# Cached PJRT runner (compile once, execute many)
# ----------------------------------------------------------------------------

_STATE = {}


def _get_runner(reps=1):
    key = (reps, VERSION, D_A, D_B, D_C, SB_BUFS)
    if key in _STATE:
        return _STATE[key]

    import jax
    from jax.experimental.shard_map import shard_map
    from jax.sharding import Mesh, PartitionSpec
    from concourse import bass2jax, mybir

    nc = build_nc(M, N, reps=reps)
    bass2jax.install_neuronx_cc_hook()

    in_names, out_names, out_avals = [], [], []
    for alloc in nc.m.functions[0].allocations:
        if not isinstance(alloc, mybir.MemoryLocationSet):
            continue
        name = alloc.memorylocations[0].name
        if alloc.kind == "ExternalInput":
            in_names.append(name)
        elif alloc.kind == "ExternalOutput":
            out_names.append(name)
            out_avals.append(jax.core.ShapedArray(
                tuple(alloc.tensor_shape), mybir.dt.np(alloc.dtype)))
    n_params = len(in_names)
    n_outs = len(out_names)
    all_in_names = tuple(in_names + out_names)
    donate = tuple(range(n_params, n_params + n_outs))

    def _jbody(*args):
        outs = bass2jax._bass_exec_p.bind(
            *args,
            out_avals=tuple(out_avals),
            in_names=all_in_names,
            out_names=tuple(out_names),
            lowering_input_output_aliases=(),
            sim_require_finite=True,
            sim_require_nnan=True,
            nc=nc,
        )
        return tuple(outs)

    devices = jax.devices()[:NCORES]
    mesh = Mesh(np.asarray(devices), ("core",))
    in_specs = (PartitionSpec("core"),) * (n_params + n_outs)
    out_specs = (PartitionSpec("core"),) * n_outs
    fn = jax.jit(
        shard_map(_jbody, mesh=mesh, in_specs=in_specs, out_specs=out_specs,
                  check_rep=False),
        keep_unused=True,
    )
    # Output staging buffers live on-device and are NOT donated, so they
    # upload once and are reused by every call (donating them would force
    # a ~26MB/core host->device refill per call).
    from jax.sharding import NamedSharding
    zeros_dev = [
        jax.device_put(
            np.zeros((NCORES * av.shape[0], *av.shape[1:]), av.dtype),
            NamedSharding(mesh, PartitionSpec("core")))
        for av in out_avals
    ]
    st = dict(fn=fn, nc=nc, in_names=in_names, out_names=out_names,
              out_avals=out_avals, n_params=n_params, zeros_dev=zeros_dev)
    _STATE[key] = st
    return st


def run_device(L, R, reps=1, _retry=True, fetch=True):
    """L, R: [NCORES, 18, M] bf16. Returns dict of full-shape outputs
    (rowneg [NCORES,128,32] fp32, colneg [NCORES,128,DEV] bf16,
    ship [NCORES,128,nblk*SHIP] bf16) holding negsq maxima / raw negsq;
    host still does the cross-partition maxes."""
    st = _get_runner(reps)
    concat_in = []
    for name in st["in_names"]:
        arr = L if name == "lhs_aug" else R
        concat_in.append(np.concatenate([arr[c] for c in range(NCORES)], axis=0))
    try:
        out_arrs = st["fn"](*concat_in, *st["zeros_dev"])
        if not fetch:
            for a in out_arrs:
                a.block_until_ready()
            return None
        out_np = [np.asarray(a) for a in out_arrs]
    except Exception:
        # The shared axon terminal occasionally reports a transient
        # device-unrecoverable state; it clears after a short pause.
        if not _retry:
            raise
        import time as _time
        _time.sleep(20.0)
        return run_device(L, R, reps=reps, _retry=False, fetch=fetch)
    outs = {}
    for i, name in enumerate(st["out_names"]):
        av = st["out_avals"][i]
        outs[name] = out_np[i].reshape(NCORES, *av.shape)
    return outs


# ----------------------------------------------------------------------------
# Public entry point
# ----------------------------------------------------------------------------

def _host_reduce(outs):
    # rowneg: [B, 128, nblk] fp32 -- fwd partial over device cols.
    # colneg: [B, 128, DEV] bf16 -- bwd partial (needs cross-partition max).
    # ship:   [B, 128, nblk*SHIP] bf16 -- raw negsq for shipped cols.
    # All hold -sq so min-sq = -max.
    nblk = M // PB
    rowneg = outs["rowneg"].astype(np.float32)            # [B,128,nblk]
    colneg = outs["colneg"].astype(np.float32)            # [B,128,DEV]
    ship = outs["ship"].reshape(B, PB, nblk, SHIP)        # bf16 view

    # fwd: per row (i, p): max over shipped cols + device partial.
    ship_f32 = np.asarray(ship, dtype=np.float32)
    fwd_ship = ship_f32.max(axis=3)                       # [B,128,nblk]
    rowmax = np.maximum(fwd_ship, rowneg)                 # [B,128,nblk]
    rowsq = np.maximum(-rowmax.astype(np.float64), 0.0)
    fwd = np.sqrt(rowsq).mean()

    # bwd: device cols [0, DEV) reduce over p; shipped cols over all (p, i).
    bwd_dev = colneg.max(axis=1)                          # [B,DEV]
    bwd_ship = ship_f32.max(axis=(1, 2))                  # [B,SHIP]
    colmax = np.concatenate([bwd_dev, bwd_ship], axis=1)  # [B,N]
    colsq = np.maximum(-colmax.astype(np.float64), 0.0)
    bwd = np.sqrt(colsq).mean()

    total = np.float32(fwd + bwd)
    return total


def kernel(pc_src, pc_dst):
    L, R = prep_inputs(pc_src, pc_dst)
    outs = run_device(L, R)
    total = _host_reduce(outs)
    return (total, total, total)
